# revision 4
# baseline (speedup 1.0000x reference)
"""Trainium2 Bass kernel for nn_CCELoss (calibration-histogram loss).

Sharding: data-parallel over image rows, 8 NeuronCores, 128 rows each.

Per-core layout: logits fp16 [114 = 6 pixel-groups x 19 classes, F=45056]
(group g covers core-flat pixels [g*F, (g+1)*F); tail of group 5 is padding
with logit 0 -> p = bf16(1/19) -> bin 0, corrected on host).

Per slice (tile 0 is sub-split 512/512/1024/2048 to shorten pipeline fill;
tiles 1..10 are full 4096):
  ACT  e16 = Exp(l16)                              bf16 out
  PE   Z[g*nq+qi, s] = sum_c e16[(g,c), qi*512+s]  (nq blockdiag matmuls,
       PSUM-accumulated into one [6*nq, 512] tile)
  ACT  Z -> SBUF f32 (PSUM egress)
  GPS  r = 1 / Z                                   bf16 out
  DMA  r -> DRAM scratch (flat == [6, width] g-major) -> replicate to
       rb16 [114, width] via a zero-stride DRAM-side access pattern
  GPS  p16 = e16 * rb16                            bf16
  folds (19 per slice, one accum column each):
    DVE  R_0 = sum max(p,0) = sum p; M_i = sum min(p, t_i) (i=4..9);
         N_i = sum [p > t_i] (i=1..9)   (tensor_scalar, 4x mode)
    ACT  R_i (i=1..3) via Relu with bias -t_i
True-class side channel (tiles 1..10): host packs gathered true-class
logits l* fp16 in [48 = g*8+q, 512] per-tile layout; e* = Exp(l*),
p* = e* * r48 is bit-identical to p16 at the true class. Tile 0's r is
exported (rout0) and its p* is computed on host. Host bins p* vs target.
Host decode: R_i = R_0 - M_i per cell, S_i = R_i + t_i*N_i,
conf_b = S_b - S_{b+1}, cnt_b = N_b - N_{b+1}, then the loss formula.
"""

import numpy as np
import ml_dtypes

import bass_rust
import concourse.bass as bass
from concourse import bacc
import concourse.mybir as mybir
import concourse.tile as tile
from concourse.vector_clock import ScopedClock
from concourse.bass_utils import run_bass_kernel_spmd

F32 = mybir.dt.float32
F16 = mybir.dt.float16
BF16 = mybir.dt.bfloat16
AF = mybir.ActivationFunctionType
ALU = mybir.AluOpType

# ---------------- problem geometry (hardcoded) ----------------
C = 19
NB = 10
H, W = 1024, 2048
NCORES = 8
ROWS = H // NCORES          # 128
NPIX = ROWS * W             # 262144 valid pixels per core
G = 6
P = G * C                   # 114 partitions
TILE_F = 4096
NT = 11
F = NT * TILE_F             # 45056
NPAD = G * F - NPIX         # 8192 pad pixels (all in group 5 tail)
VALID_J5 = NPIX - 5 * F     # 36864 valid pixels in group 5
PAD_TILE0 = VALID_J5 // TILE_F  # = 9; tiles 9,10 have group 5 all-pad

NQ = 8                      # 512-wide chunks per full tile
QW = TILE_F // NQ           # 512
PCOLS = NT * QW             # 5632 pstar columns (tile 0 region host-filled)

# slices: (tile, col0, width); tile 0 split to cut pipeline fill latency
SLICES = [(0, 0, 512), (0, 512, 512), (0, 1024, 1024), (0, 2048, 2048)] + \
         [(t, 0, TILE_F) for t in range(1, NT)]
NS = len(SLICES)
# stationary const blocks per chunk-count nq: offset into bd16 columns
NQOFF = {8: 0, 4: NQ * 48, 2: NQ * 48 + 4 * 24, 1: NQ * 48 + 4 * 24 + 2 * 12}
BDW = NQ * 48 + 4 * 24 + 2 * 12 + 1 * 6   # 510

THR = [np.float32(i / 10.0) for i in range(10)]
NFOLD = 19                  # slots: 0..9 = R_0/M_i, 10..18 = N_1..N_9
ACT_CONF = (1, 2, 3)        # conf folds on ACT (true R_i via Relu)

_BUILD_CACHE = {}


def _patch_tile_drain():
    """walrus rejects drains with >1 sync wait; split the tile-exit drain."""
    if getattr(tile.TileContext, "_drain_patched", False):
        return

    def _drain_and_barrier(self, tick_clock, wait_clock):
        drain_inst = self.nc.sync.drain()
        wait_clock.add_sem_waits(
            drain_inst.ins, ScopedClock({None: tick_clock.global_clock})
        )
        si = drain_inst.ins.sync_info
        if si is not None and len(si.on_wait) > 1:
            waits = list(si.on_wait)
            ups = list(si.on_update)
            drain_inst.ins.sync_info = mybir.SyncInfo(on_wait=waits[:1], on_update=[])
            last = drain_inst
            for i in range(1, len(waits)):
                last = self.nc.sync.drain()
                last.ins.sync_info = mybir.SyncInfo(on_wait=waits[i:i + 1], on_update=[])
            if ups:
                lw = list(last.ins.sync_info.on_wait) if last.ins.sync_info else []
                last.ins.sync_info = mybir.SyncInfo(on_wait=lw, on_update=ups)
        self.nc.all_engine_barrier()
        assert self.sems is not None
        popped = self.nc._tile_sem_poison_stack.pop()
        assert popped is self._sem_poison
        self.nc.clear_and_free_semaphores(list(self.sems.allocated().values()))
        self.nc.all_engine_barrier()

    tile.TileContext._drain_and_barrier = _drain_and_barrier
    tile.TileContext._drain_patched = True


def build_nc():
    _patch_tile_drain()
    nc = bacc.Bacc()

    # threshold constants as const APs for ACT bias
    for i in range(1, 10):
        for v in (float(-THR[i]), float(THR[i])):
            if (F32, v) in nc.const_aps.aps:
                continue
            tns = nc.alloc_sbuf_tensor(f"const-thr-{v}", [128, 1], F32)
            nc.gpsimd.memset(tns.ap(), v)
            nc.const_aps.aps[(F32, v)] = tns.ap()
    nc.all_engine_barrier()

    lg = nc.declare_dram_parameter("lg", [C, NPIX], F16, isOutput=False)
    zpad = nc.declare_dram_parameter("zpad", [C, TILE_F], F16, isOutput=False)
    lstar = nc.declare_dram_parameter("lstar", [48, PCOLS], F16, isOutput=False)
    bd16 = nc.declare_dram_parameter("bd16", [P, BDW], BF16, isOutput=False)
    folds_out = nc.declare_dram_parameter("folds", [P, NS * NFOLD], F32, isOutput=True)
    pstar_out = nc.declare_dram_parameter("pstar", [48, PCOLS], BF16, isOutput=True)
    rout0 = nc.declare_dram_parameter("rout0", [2 * G, TILE_F], BF16, isOutput=True)

    with tile.TileContext(nc) as tc:
        with (
            tc.tile_pool(name="const", bufs=1) as constp,
            tc.tile_pool(name="lt", bufs=3) as lp,
            tc.tile_pool(name="et", bufs=3) as ep,
            tc.tile_pool(name="rbt", bufs=3) as rbp,
            tc.tile_pool(name="pt", bufs=3) as pp,
            tc.tile_pool(name="zf", bufs=3) as rfp,
            tc.tile_pool(name="r48", bufs=3) as rsp,
            tc.tile_pool(name="lst", bufs=2) as lsp,
            tc.tile_pool(name="est", bufs=2) as esp,
            tc.tile_pool(name="acc", bufs=1) as accp,
            tc.tile_pool(name="zpsum", bufs=3, space="PSUM") as zp,
            tc.tile_pool(name="rdram", bufs=3, space="DRAM") as rdp,
        ):
            bd_sb = constp.tile([P, BDW], BF16)
            nc.sync.dma_start(out=bd_sb[:], in_=bd16[:])
            ones48 = constp.tile([48, QW], BF16)
            nc.gpsimd.memset(ones48[:], 1.0)

            foldacc = accp.tile([P, NS * NFOLD], F32)
            pstar_sb = accp.tile([48, PCOLS], BF16)
            scr_dve = accp.tile([P, TILE_F], BF16)
            scr_act = accp.tile([P, TILE_F], BF16)

            pts = {}
            for ss in range(NS + 1):
              if ss < NS:
                t, col0, width = SLICES[ss]
                nq = width // QW

                # ---- load logits slice [114, width] fp16 ----
                lt = lp.tile([P, width], F16)
                ng = G if t < PAD_TILE0 else G - 1
                base = lg[:, t * TILE_F + col0:t * TILE_F + col0 + width]
                src3 = bass_rust.AP(
                    tensor=base.tensor, offset=base.offset,
                    ap=[[F, ng]] + list(base.ap))
                nc.scalar.dma_start(out=lt[0:C * ng, :], in_=src3)
                if ng < G:
                    nc.scalar.dma_start(out=lt[C * 5:P, :], in_=zpad[:, 0:width])

                # ---- e = exp(l) -> bf16 ----
                et = ep.tile([P, width], BF16)
                nc.scalar.activation(et[:], lt[:], AF.Exp)

                # ---- Z[g*nq+qi, s] = sum_c e[(g,c), qi*512+s] ----
                zps = zp.tile([6 * nq, QW], F32)
                off = NQOFF[nq]
                for qi in range(nq):
                    nc.tensor.matmul(
                        zps[:],
                        bd_sb[:, off + qi * 6 * nq:off + (qi + 1) * 6 * nq],
                        et[:, qi * QW:(qi + 1) * QW],
                        start=(qi == 0), stop=(qi == nq - 1),
                    )

                # ---- r = 1/Z (DVE fp32 approx), downcast on ACT ----
                rf = rfp.tile([6 * nq, QW], F32)
                nc.vector.reciprocal_approx_fast(rf[:], zps[:])
                r48 = rsp.tile([6 * nq, QW], BF16)
                nc.scalar.copy(r48[:], rf[:])

                # ---- bounce through DRAM; replicate to [114, width] ----
                # r48 partition-major dump [6*nq, 512] == [6, width] g-major
                if width < TILE_F:
                    blk = 0 if t == 0 else G
                    rdap = rout0[blk:blk + G, col0:col0 + width]
                    rdsrc = bass_rust.AP(
                        tensor=rdap.tensor, offset=rdap.offset,
                        ap=[[TILE_F, G], [0, C], [1, width]])
                else:
                    rd = rdp.tile([6 * nq, QW], BF16)
                    rdfl = rd[:]
                    rdap = bass_rust.AP(
                        tensor=rdfl.tensor, offset=rdfl.offset,
                        ap=[[width, G], [1, width]])
                    rdsrc = bass_rust.AP(
                        tensor=rdfl.tensor, offset=rdfl.offset,
                        ap=[[width, G], [0, C], [1, width]])
                nc.sync.dma_start(out=rdap, in_=r48[:])
                rbt = rbp.tile([P, width], BF16)
                nc.sync.dma_start(out=rbt[:], in_=rdsrc)

                # ---- p = e * r (gpsimd) ----
                pt = pp.tile([P, width], BF16)
                nc.gpsimd.tensor_tensor(pt[:], et[:], rbt[:], ALU.mult)

                # ---- true-class side channel (full tiles only) ----
                if width == TILE_F:
                    lst = lsp.tile([48, QW], F16)
                    nc.scalar.dma_start(
                        out=lst[:], in_=lstar[:, t * QW:(t + 1) * QW])
                    est = esp.tile([48, QW], BF16)
                    nc.scalar.activation(est[:], lst[:], AF.Exp)
                    # rstar[g*8+q, s] = r of pixel (g, q*512+s): from rd flat
                    rstar = rsp.tile([48, QW], BF16)
                    rssrc = bass_rust.AP(
                        tensor=rdfl.tensor, offset=rdfl.offset,
                        ap=[[TILE_F, G], [QW, NQ], [1, QW]])
                    nc.sync.dma_start(out=rstar[:], in_=rssrc)
                    nc.gpsimd.tensor_tensor(
                        pstar_sb[:, t * QW:(t + 1) * QW], est[:], rstar[:],
                        ALU.mult)

                pts[ss] = pt
              if ss >= 1:
                s0 = ss - 1
                t, col0, width = SLICES[s0]
                pt = pts.pop(s0)
                act_conf = (1, 2, 3, 4) if s0 == NS - 1 else ACT_CONF
                # ---- folds ----
                base_c = s0 * NFOLD
                nc.vector.tensor_scalar(
                    scr_dve[:, 0:width], pt[:], 0.0, None, ALU.max, ALU.add,
                    accum_out=foldacc[:, base_c:base_c + 1])
                for i in range(1, 10):
                    col_r = foldacc[:, base_c + i:base_c + i + 1]
                    col_n = foldacc[:, base_c + 9 + i:base_c + 10 + i]
                    ti = float(THR[i])
                    if i in act_conf:
                        nc.scalar.activation(
                            scr_act[:, 0:width], pt[:], AF.Relu, bias=-ti,
                            accum_out=col_r)
                    else:
                        # M_i = sum min(p, t_i); decode: R_i = R_0 - M_i
                        nc.vector.tensor_scalar(
                            scr_dve[:, 0:width], pt[:], ti, None, ALU.min,
                            ALU.add, accum_out=col_r)
                    nc.vector.tensor_scalar(
                        scr_dve[:, 0:width], pt[:], ti, None, ALU.is_gt,
                        ALU.add, accum_out=col_n)

            # ---- end phase ----
            nc.sync.dma_start(out=pstar_out[:], in_=pstar_sb[:])
            nc.sync.dma_start(out=folds_out[:], in_=foldacc[:])

    nc.finalize()
    return nc


def _make_consts():
    # stationary for chunk qi at chunk-count nq: S[(g,c), g*nq+qi] = 1,
    # laid out at column offset NQOFF[nq] + qi*6*nq
    bd = np.zeros((P, BDW), np.float32)
    for nq in (8, 4, 2, 1):
        off = NQOFF[nq]
        for qi in range(nq):
            for g in range(G):
                bd[C * g:C * (g + 1), off + qi * 6 * nq + g * nq + qi] = 1.0
    return bd.astype(ml_dtypes.bfloat16)


def _shard_host(output: np.ndarray, target: np.ndarray):
    o = np.ascontiguousarray(output[0])          # [19, 1024, 2048] f32
    t = np.ascontiguousarray(target[0])          # [1024, 2048] int32
    lstar_full = np.take_along_axis(o, t[None], axis=0)[0]
    bd = _make_consts()
    zpad = np.zeros((C, TILE_F), np.float16)

    in_maps = []
    for core in range(NCORES):
        r0 = core * ROWS
        lg = o[:, r0:r0 + ROWS, :].reshape(C, NPIX).astype(np.float16)
        ls = lstar_full[r0:r0 + ROWS, :].reshape(-1).astype(np.float16)
        ls = np.concatenate([ls, np.zeros(NPAD, np.float16)])
        # flat n = g*F + t*4096 + q*512 + s  ->  [48 = g*8+q, t*512 + s]
        ls = (ls.reshape(G, NT, NQ, QW).transpose(0, 2, 1, 3)
                .reshape(48, PCOLS))
        in_maps.append({
            "lg": np.ascontiguousarray(lg),
            "lstar": np.ascontiguousarray(ls),
            "bd16": bd, "zpad": zpad,
        })
    return in_maps


def _pstar_to_flat(ps: np.ndarray) -> np.ndarray:
    """[48 = g*8+q, PCOLS] -> core-flat [G*F] (incl pad)."""
    return (ps.reshape(G, NQ, NT, QW).transpose(0, 2, 1, 3).reshape(-1))


def _decode_and_loss(results, in_maps, target: np.ndarray):
    BF = ml_dtypes.bfloat16
    conf = np.zeros((C, NB), np.float64)
    cnt = np.zeros((C, NB), np.float64)
    acc = np.zeros((C, NB), np.float64)

    p_pad = float(np.float32(1.0 / np.float32(19.0)).astype(BF))

    for core in range(NCORES):
        folds = np.asarray(results[core]["folds"], np.float64)
        folds = folds.reshape(P, NS, NFOLD).copy()
        # DVE conf slots hold M_i = sum min(p, t_i); R_i = R_0 - M_i per cell
        for i in range(1, 10):
            if i in ACT_CONF:
                continue
            sl = [s for s in range(NS) if not (s == NS - 1 and i == 4)]
            folds[:, sl, i] = folds[:, sl, 0] - folds[:, sl, i]
        folds = folds.sum(axis=1)                                  # [114, 19]
        R = folds[:, :10].reshape(G, C, 10).sum(axis=0)            # [C, 10]
        Ni = folds[:, 10:].reshape(G, C, 9).sum(axis=0)            # [C, 9]

        R[:, 0] -= NPAD * np.float64(p_pad)     # pads contribute only to R_0
        Ni = np.concatenate([np.full((C, 1), float(NPIX)), Ni], axis=1)

        tgrid = np.arange(10, dtype=np.float64) / 10.0
        S = R + tgrid[None, :] * Ni             # S_i = sum p * [p > t_i]
        Snext = np.concatenate([S[:, 1:], np.zeros((C, 1))], axis=1)
        Nnext = np.concatenate([Ni[:, 1:], np.zeros((C, 1))], axis=1)
        conf += S - Snext
        cnt += Ni - Nnext

        # pstar: device for full tiles; tiles 0 and 10 from host * rout0
        ps = np.asarray(results[core]["pstar"], np.float32)
        rout = np.asarray(results[core]["rout0"], np.float32)  # [12, 4096]
        ls48 = np.asarray(in_maps[core]["lstar"], np.float32)  # [48, PCOLS]
        for blk, t in ((0, 0),):
            estar = np.exp(ls48[:, t * QW:(t + 1) * QW]
                           ).astype(BF).astype(np.float32)
            r48h = rout[blk * G:(blk + 1) * G].reshape(G, NQ, QW
                                                       ).reshape(48, QW)
            ps[:, t * QW:(t + 1) * QW] = (estar * r48h
                                          ).astype(BF).astype(np.float32)

        r0 = core * ROWS
        psf = _pstar_to_flat(ps)[:NPIX]
        y = target[0, r0:r0 + ROWS, :].reshape(-1)
        b = np.clip(np.ceil(psf * np.float32(10.0)).astype(np.int32) - 1,
                    0, NB - 1)
        acc += np.bincount(y * NB + b, minlength=C * NB).reshape(C, NB)

    EPS = 1e-13
    avg_acc = acc / (cnt + EPS)
    avg_conf = conf / (cnt + EPS)
    loss = np.sum((avg_acc - avg_conf) ** 2 * (cnt / cnt.sum()))
    return np.float32(loss), (conf, cnt, acc)


def kernel(output: np.ndarray, target: np.ndarray) -> np.ndarray:
    output = np.asarray(output, np.float32)
    target = np.asarray(target, np.int32)
    if "nc" not in _BUILD_CACHE:
        _BUILD_CACHE["nc"] = build_nc()
    nc = _BUILD_CACHE["nc"]
    in_maps = _shard_host(output, target)
    res = run_bass_kernel_spmd(nc, in_maps, list(range(NCORES)))
    loss, _ = _decode_and_loss(res.results, in_maps, target)
    return np.float32(loss)
